# revision 39
# baseline (speedup 1.0000x reference)
"""Trainium2 kernel for nn_HSCR_67396626809127 (gnn_message_passing).

The reference network (fc1/fc2 -> 24-step KTD kinematic-tree recurrence ->
cam/pose/shape heads) contains no nonlinearity (dropout is identity in eval
mode), so the whole module is one affine map:

    out[157] = W @ [x(256) | init_pose(144) | init_shape(10) | init_cam(3)] + b

W [157,413] / b [157] are composed on host in float64 from the small weight
tensors, with the bias folded in as a constant-ones feature row (K padded to
416 = 3x128 + 32).  The device runs a data-parallel bf16 matmul over the
B*T = 32768 tokens: each of the 8 cores handles 4096 tokens.

Schedule notes (from profile iterations):
  - bf16 end-to-end: halves HBM bytes; PE streams 1 col/cycle with FWL
  - k-inner accumulation into one PSUM bank sustains 216 ns per N=512 matmul;
    bank-alternating orders lose fill/drain overlap (379 ns)
  - warm-up matmuls on a zeroed SBUF tile (no DMA dependency) keep the PE HAM
    clock gate warm (2.4 GHz) until real data arrives
  - inputs stay on 128 partitions with >=6KB contiguous runs per partition
    (104-partition or 4KB-run layouts drop aggregate DMA from ~400 to ~240)
  - the 29-row output block is computed into rotating 32-partition column
    bands (tile_position) of a shared PSUM bank, so it leaves as fast
    128-partition stores; 29-partition DMAs dribble at ~20 GB/s
"""

import numpy as np
import ml_dtypes

BF16 = ml_dtypes.bfloat16

ANCESTOR_INDEX = [[], [0], [0], [0], [0, 1], [0, 2], [0, 3], [0, 1, 4],
                  [0, 2, 5], [0, 3, 6], [0, 1, 4, 7], [0, 2, 5, 8],
                  [0, 3, 6, 9], [0, 3, 6, 9], [0, 3, 6, 9], [0, 3, 6, 9, 12],
                  [0, 3, 6, 9, 13], [0, 3, 6, 9, 14], [0, 3, 6, 9, 13, 16],
                  [0, 3, 6, 9, 14, 17], [0, 3, 6, 9, 13, 16, 18],
                  [0, 3, 6, 9, 14, 17, 19], [0, 3, 6, 9, 13, 16, 18, 20],
                  [0, 3, 6, 9, 14, 17, 19, 21]]
HID = 1024
NCORES = 8
B, T = 2048, 16
NTOK = B * T                 # 32768
TPC = NTOK // NCORES         # 4096 tokens per core
NOUT = 157                   # [cam 3 | pose 144 | shape 10]
KV = 416                     # 413 features + ones row (bias) + 2 zero pad
TW = 1024                    # tokens per SBUF tile
NT = TPC // TW               # 4 tiles per core
NWARM = 14                   # PE warm-up matmuls (HAM clock gate)
WCOL = 3 * TW                # column offset of weights inside tile-0's load
T0W = 3 * TW + 4 * 160       # tile-0 SBUF width: activations + weights

_PROG = {}


def _compose_affine(fc1_w, fc1_b, fc2_w, fc2_b, decshape_w, decshape_b,
                    deccam_w, deccam_b, ktd_w, ktd_b):
    """Fold the whole network into out = v @ W.T + b, v = [x|pose|shape|cam]."""
    f8 = np.float64
    fc1_w, fc1_b = fc1_w.astype(f8), fc1_b.astype(f8)
    fc2_w, fc2_b = fc2_w.astype(f8), fc2_b.astype(f8)
    decshape_w, decshape_b = decshape_w.astype(f8), decshape_b.astype(f8)
    deccam_w, deccam_b = deccam_w.astype(f8), deccam_b.astype(f8)
    ktd_w, ktd_b = ktd_w.astype(f8), ktd_b.astype(f8)

    F1x, F1s = fc1_w[:, :256], fc1_w[:, 256:266]
    F2x, F2p = fc2_w[:, :256], fc2_w[:, 256:400]

    # KTD recurrence -> pose_out = G @ xc_pose + H @ init_pose + c
    G = np.zeros((24, 6, HID)); H = np.zeros((24, 6, 144)); c = np.zeros((24, 6))
    for j, anc in enumerate(ANCESTOR_INDEX):
        Wj = ktd_w[j]
        G[j] = Wj[:, :HID]
        off = HID
        for i in anc:
            A = Wj[:, off:off + 6]; off += 6
            G[j] += A @ G[i]
            H[j] += A @ H[i]
            c[j] += A @ c[i]
        # reference concatenates init_pose[..., j:j+6] (overlapping slice)
        H[j][:, j:j + 6] += Wj[:, off:off + 6]
        c[j] += ktd_b[j]
    G = G.reshape(144, HID); H = H.reshape(144, 144); c = c.reshape(144)

    Dp, Ds, Dc = deccam_w[:, :HID], deccam_w[:, HID:2 * HID], deccam_w[:, 2 * HID:]

    W = np.zeros((NOUT, 413)); b = np.zeros(NOUT)
    W[0:3, 0:256] = Dp @ F2x + Ds @ F1x
    W[0:3, 256:400] = Dp @ F2p
    W[0:3, 400:410] = Ds @ F1s
    W[0:3, 410:413] = Dc + np.eye(3)
    b[0:3] = Dp @ fc2_b + Ds @ fc1_b + deccam_b

    W[3:147, 0:256] = G @ F2x
    W[3:147, 256:400] = G @ F2p + H + np.eye(144)
    b[3:147] = G @ fc2_b + c

    W[147:157, 0:256] = decshape_w @ F1x
    W[147:157, 400:410] = decshape_w @ F1s + np.eye(10)
    b[147:157] = decshape_w @ fc1_b + decshape_b
    return W.astype(np.float32), b.astype(np.float32)


def _build_program():
    import concourse.bass as bass
    import concourse.tile as tile
    from concourse import bacc, mybir

    f32 = mybir.dt.float32
    bf16 = mybir.dt.bfloat16
    nc = bacc.Bacc("TRN2", target_bir_lowering=False, debug=False,
                   num_devices=NCORES)
    # tile 0 + weights in ONE fat transfer (7.4KB contiguous per partition):
    # a separate small-descriptor weight DMA dribbles at ~40 GB/s under load
    vin0 = nc.declare_dram_parameter("vin0", [128, T0W], bf16, isOutput=False)
    # k-chunks 0..2 (features 0..383) of tiles 1..3: 6KB contiguous runs
    vin = nc.declare_dram_parameter("vin", [128, NT - 1, 3, TW], bf16,
                                    isOutput=False)
    # k-chunk 3 (features 384..415): whole core upfront
    v32 = nc.declare_dram_parameter("v32", [32, TPC], bf16, isOutput=False)
    o0 = nc.declare_dram_parameter("o0", [128, NT // 2, 2 * TW], bf16,
                                   isOutput=True)
    # M1 output in column-band layout: [half, band-partition, 512 tokens]
    o1 = nc.declare_dram_parameter("o1", [2, 128, 512], bf16, isOutput=True)

    with tile.TileContext(nc) as tc:
        with (
            tc.tile_pool(name="wpool", bufs=1) as wpool,
            tc.tile_pool(name="rhs", bufs=NT) as rpool,
            tc.tile_pool(name="outp", bufs=2) as opool,
            tc.tile_pool(name="ps0", bufs=3, space=bass.MemorySpace.PSUM) as p0pool,
            tc.tile_pool(name="ps1", bufs=2, space=bass.MemorySpace.PSUM) as p1pool,
            tc.tile_pool(name="warm", bufs=2, space=bass.MemorySpace.PSUM) as wmpool,
        ):
            # zeroed tile lets warm-up matmuls run before any DMA lands
            zt = wpool.tile([128, 512], bf16, tag="zt", name="zt")
            nc.vector.memset(zt[:], 0.0)
            for i in range(NWARM):
                pw = wmpool.tile([128, 512], f32, tag="wm", name=f"wm_{i}")
                nc.tensor.matmul(pw[:], zt[:, 0:128], zt[:],
                                 start=True, stop=True)

            # ring order is load-bearing: HWDGE rings are FIFO and the first
            # tile must win the bandwidth race; SWDGE stays OFF during the
            # input phase (Q7 descriptor rings contend for SBUF AXI ports)
            rt0 = wpool.tile([128, T0W], bf16, tag="r0", name="r0")
            v3s = wpool.tile([32, TPC], bf16, tag="v3", name="v3")
            rtiles = [rpool.tile([128, 3, TW], bf16, tag="r", name=f"r_{t}")
                      for t in range(1, NT)]
            # rt0 leads sync; v32 (small, needed by the first k3 matmul)
            # rides sync right behind it; r1 leads scalar so tile 1 arrives
            # ~2us earlier than when queued behind v32
            nc.sync.dma_start(rt0[:], vin0[:])
            nc.scalar.dma_start(rtiles[0][:], vin[:, 0])
            nc.sync.dma_start(v3s[:], v32[:])
            nc.sync.dma_start(rtiles[1][:], vin[:, 1])
            nc.scalar.dma_start(rtiles[2][:], vin[:, 2])

            def wsl(k, m0, dm, kp=128):
                c = WCOL + k * 160 + m0
                return rt0[0:kp, c:c + dm]

            def rsl(t, k, h):
                if t == 0:
                    c = k * TW + h * 512
                    return rt0[:, c:c + 512]
                return rtiles[t - 1][:, k, bass.ts(h, 512)]

            ps1x = None
            for t in range(NT):
                if t % 2 == 0:
                    ot0 = opool.tile([128, 2 * TW], bf16, tag="o0",
                                     name=f"o0_{t // 2}")
                for h in range(2):
                    cs = bass.ts(2 * t + h, 512)
                    idx = 2 * t + h
                    bnd, half = idx % 4, idx // 4

                    # M0: full-width, k-inner accumulation into one bank
                    ps = p0pool.tile([128, 512], f32, tag="ps0",
                                     name=f"ps0_{t}_{h}")
                    for k in range(3):
                        nc.tensor.matmul(ps[:], wsl(k, 0, 128), rsl(t, k, h),
                                         start=(k == 0), stop=False)
                    nc.tensor.matmul(ps[:], wsl(3, 0, 128, kp=32), v3s[:, cs],
                                     start=False, stop=True)
                    nc.vector.tensor_copy(ot0[:, bass.ts(2 * (t % 2) + h, 512)],
                                          ps[:])

                    # M1: 32 outputs into rotating column band of shared bank
                    if bnd == 0:
                        ps1x = p1pool.tile([128, 512], f32, tag="ps1",
                                           name=f"ps1_{half}")
                    pb = ps1x[32 * bnd:32 * bnd + 32, :]
                    tp = (0, 32 * bnd)
                    for k in range(3):
                        nc.tensor.matmul(pb, wsl(k, 128, 32), rsl(t, k, h),
                                         start=(k == 0), stop=False,
                                         tile_position=tp)
                    nc.tensor.matmul(pb, wsl(3, 128, 32, kp=32), v3s[:, cs],
                                     start=False, stop=True, tile_position=tp)
                    if bnd == 3:
                        ot1 = opool.tile([128, 512], bf16, tag="o1",
                                         name=f"o1_{half}")
                        nc.vector.tensor_copy(ot1[:], ps1x[:])
                        nc.scalar.dma_start(o1[half], ot1[:])

                if t % 2 == 1:
                    nc.sync.dma_start(o0[:, t // 2], ot0[:])
    nc.compile()
    return nc


def _get_program():
    if "nc" not in _PROG:
        _PROG["nc"] = _build_program()
    return _PROG["nc"]


def _make_in_maps(x, init_pose, init_shape, init_cam, fc1_w, fc1_b, fc2_w,
                  fc2_b, decshape_w, decshape_b, deccam_w, deccam_b, ktd_w,
                  ktd_b):
    x = np.asarray(x, dtype=np.float32)
    init_pose = np.asarray(init_pose, dtype=np.float32)
    init_shape = np.asarray(init_shape, dtype=np.float32)
    init_cam = np.asarray(init_cam, dtype=np.float32)

    W, b = _compose_affine(
        np.asarray(fc1_w), np.asarray(fc1_b), np.asarray(fc2_w),
        np.asarray(fc2_b), np.asarray(decshape_w), np.asarray(decshape_b),
        np.asarray(deccam_w), np.asarray(deccam_b), np.asarray(ktd_w),
        np.asarray(ktd_b))
    # bias as ones-row feature; K padded 414 -> 416 = 3x128 + 32
    W_aug = np.zeros((NOUT, KV), np.float32)
    W_aug[:, 0:413] = W
    W_aug[:, 413] = b
    # M dim padded 157 -> 160: cols 0..127 = M0, 128..156 = M1
    wtk = np.zeros((KV, 160), np.float32)
    wtk[:, 0:NOUT] = W_aug.T
    wtb = wtk.astype(BF16)
    wt = np.zeros((128, 4, 160), BF16)
    for ci in range(3):
        wt[:, ci, :] = wtb[ci * 128:(ci + 1) * 128]
    wt[0:32, 3, :] = wtb[384:416]
    wflat = wt.reshape(128, 4 * 160)

    xs = x.reshape(NCORES, TPC, 256)
    ps = init_pose.reshape(NCORES, TPC, 144)
    ss = init_shape.reshape(NCORES, TPC, 10)
    cs = init_cam.reshape(NCORES, TPC, 3)

    in_maps = []
    for i in range(NCORES):
        v = np.zeros((KV, TPC), np.float32)                 # feature-major shard
        v[0:256] = xs[i].T
        v[256:400] = ps[i].T
        v[400:410] = ss[i].T
        v[410:413] = cs[i].T
        v[413] = 1.0
        vb = v.astype(BF16)
        # vin[p, t-1, c, w] = v[c*128+p, t*TW+w] for tiles 1..3
        vall = vb[0:384].reshape(3, 128, NT, TW).transpose(1, 2, 0, 3)
        vin = np.ascontiguousarray(vall[:, 1:])
        # tile 0 activations + flattened weights in one tensor
        vin0 = np.concatenate(
            [vall[:, 0].reshape(128, 3 * TW), wflat], axis=1)
        in_maps.append({
            "vin0": np.ascontiguousarray(vin0),
            "vin": vin,
            "v32": np.ascontiguousarray(vb[384:416]),
        })
    return in_maps


def _assemble(results):
    out_t = np.empty((NOUT, NTOK), np.float32)
    for i in range(NCORES):
        sl = slice(i * TPC, (i + 1) * TPC)
        out_t[0:128, sl] = results[i]["o0"].reshape(128, TPC)
        # o1[half, 32*band + j, c] -> row 128+j, token (4*half+band)*512 + c
        o1 = results[i]["o1"].reshape(2, 4, 32, 512)[:, :, 0:29, :]
        out_t[128:157, sl] = o1.transpose(2, 0, 1, 3).reshape(29, TPC)
    return np.ascontiguousarray(out_t.T)


def kernel(x, init_pose, init_shape, init_cam, fc1_w, fc1_b, fc2_w, fc2_b,
           decshape_w, decshape_b, deccam_w, deccam_b, ktd_w, ktd_b):
    from concourse.bass_utils import run_bass_kernel_spmd

    in_maps = _make_in_maps(x, init_pose, init_shape, init_cam, fc1_w, fc1_b,
                            fc2_w, fc2_b, decshape_w, decshape_b, deccam_w,
                            deccam_b, ktd_w, ktd_b)
    nc = _get_program()
    res = run_bass_kernel_spmd(nc, in_maps, list(range(NCORES)))
    return _assemble(res.results)
